# revision 10
# baseline (speedup 1.0000x reference)
"""GPS (GIN + global attention) kernel for 8 TRN2 NeuronCores.

Sharding: nodes split 512/core. Features kept transposed (xT [C=128, nodes]).
- GIN aggregation = dense A.T slice matmul (A built host-side from edge_index,
  streamed per layer from HBM, f32r full-rate).
- Attention: scores transposed ST[k, q] per head (K=32 matmuls, N=512 free),
  exp on ACT (no max subtraction; scores are O(1)), denominator via ones
  column appended to V, AV accumulates oT[33, 512] over k-tiles.
- BatchNorm over nodes: per-core [sum, sumsq] + cross-core AllReduce;
  rsqrt computed as exp(-0.5*ln(var+eps)) (single ACT table set).
- Per layer: one AllReduce [128,4] (BN1+BN2 stats) and one AllGather of the
  pre-BN3 layer output with BN3 stats piggybacked; BN3 applied post-gather.
"""
import sys
sys.path.insert(0, "/opt/trn_rl_repo")

import numpy as np
import concourse.bacc as bacc
import concourse.mybir as mybir
import concourse.tile as tile
from concourse import bass_utils

N = 4096
C = 128
L = 4
E = 131072
HEADS = 4
D = C // HEADS            # 32
NCORES = 8
NO = N // NCORES          # 512 nodes per core
BN_EPS = 1e-5
RG = [[i for i in range(NCORES)]]

f32 = mybir.dt.float32
f32r = mybir.dt.float32r
AF = mybir.ActivationFunctionType
OP = mybir.AluOpType
AX = mybir.AxisListType

_CACHED = {}


def _build():
    nc = bacc.Bacc("TRN2", target_bir_lowering=False, num_devices=NCORES)

    # ---------------- DRAM I/O ----------------
    d_xT = nc.dram_tensor("xT", [C, N], f32r, kind="ExternalInput")
    d_xq = nc.dram_tensor("xq", [C, NO], f32r, kind="ExternalInput")       # per-core
    d_AT = nc.dram_tensor("AT", [N, NO], f32r, kind="ExternalInput")       # per-core
    d_ident = nc.dram_tensor("ident", [C, C], f32r, kind="ExternalInput")
    d_ones = nc.dram_tensor("onesc", [C, C], f32r, kind="ExternalInput")
    d_wiT = nc.dram_tensor("wiT", [L, C, 3 * C + 64], f32r, kind="ExternalInput")
    d_wib = nc.dram_tensor("wib", [L, C, 5], f32, kind="ExternalInput")
    d_woTh = nc.dram_tensor("woTh", [L, D, 4 * C], f32r, kind="ExternalInput")
    d_wob = nc.dram_tensor("wob", [L, C, 1], f32, kind="ExternalInput")
    d_g1T = nc.dram_tensor("g1T", [L, C, C], f32r, kind="ExternalInput")
    d_g1b = nc.dram_tensor("g1b", [L, C, 1], f32, kind="ExternalInput")
    d_g2T = nc.dram_tensor("g2T", [L, C, C], f32r, kind="ExternalInput")
    d_g2b = nc.dram_tensor("g2b", [L, C, 1], f32, kind="ExternalInput")
    d_m1T = nc.dram_tensor("m1T", [L, C, 2 * C], f32r, kind="ExternalInput")
    d_m1b = nc.dram_tensor("m1b", [L, C, 2], f32, kind="ExternalInput")
    d_m2T = nc.dram_tensor("m2T", [L, C, 2, C], f32r, kind="ExternalInput")
    d_m2b = nc.dram_tensor("m2b", [L, C, 1], f32, kind="ExternalInput")
    d_bng = nc.dram_tensor("bng", [L, 3, C, 1], f32, kind="ExternalInput")
    d_bnb = nc.dram_tensor("bnb", [L, 3, C, 1], f32, kind="ExternalInput")
    d_hw1 = nc.dram_tensor("hw1T", [C, C // 2], f32r, kind="ExternalInput")
    d_hw2 = nc.dram_tensor("hw2T", [C // 2, C // 4], f32r, kind="ExternalInput")
    d_hw3 = nc.dram_tensor("hw3T", [C // 4, 1], f32r, kind="ExternalInput")
    d_hb1 = nc.dram_tensor("hb1", [C // 2, 1], f32, kind="ExternalInput")
    d_hb2 = nc.dram_tensor("hb2", [C // 4, 1], f32, kind="ExternalInput")
    d_hb3 = nc.dram_tensor("hb3", [1, 1], f32, kind="ExternalInput")
    d_out = nc.dram_tensor("out", [1, NO], f32, kind="ExternalOutput")

    KT = N // C               # 32 k-tiles of 128 nodes
    NQC = N // 512            # 8 column chunks of 512

    with tile.TileContext(nc) as tc:
        with tc.tile_pool(name="const", bufs=1) as cp, \
             tc.tile_pool(name="big", bufs=1) as bp, \
             tc.tile_pool(name="work", bufs=1) as wp, \
             tc.tile_pool(name="stream", bufs=4) as sp, \
             tc.tile_pool(name="psum_st", bufs=3, space="PSUM") as pst, \
             tc.tile_pool(name="psum_ot", bufs=2, space="PSUM") as pot, \
             tc.tile_pool(name="psum_agg", bufs=1, space="PSUM") as pagg, \
             tc.tile_pool(name="psum_mm", bufs=2, space="PSUM") as pmm, \
             tc.tile_pool(name="dram", bufs=2, space="DRAM") as dp:

            # ---------------- constants ----------------
            ident = cp.tile([C, C], f32r, name="ident")
            nc.sync.dma_start(out=ident[:], in_=d_ident[:])
            onesc = cp.tile([C, C], f32r, name="onesc")
            nc.sync.dma_start(out=onesc[:], in_=d_ones[:])
            epsc = cp.tile([C, 1], f32, name="epsc")
            nc.vector.memset(epsc[:], BN_EPS)

            W = {}
            for i in range(L):
                for key, dt_, dram, shp in (
                    ("wiT", f32r, d_wiT, [C, 3 * C + 64]),
                    ("wib", f32, d_wib, [C, 5]),
                    ("woTh", f32r, d_woTh, [D, 4 * C]),
                    ("wob", f32, d_wob, [C, 1]),
                    ("g1T", f32r, d_g1T, [C, C]),
                    ("g1b", f32, d_g1b, [C, 1]),
                    ("g2T", f32r, d_g2T, [C, C]),
                    ("g2b", f32, d_g2b, [C, 1]),
                    ("m1T", f32r, d_m1T, [C, 2 * C]),
                    ("m1b", f32, d_m1b, [C, 2]),
                    ("m2T", f32r, d_m2T, [C, 2, C]),
                    ("m2b", f32, d_m2b, [C, 1]),
                ):
                    t = cp.tile(shp, dt_, name=f"{key}_{i}")
                    nc.sync.dma_start(out=t[:], in_=dram[i])
                    W[(key, i)] = t
                for j, key in ((0, "n1"), (1, "n2"), (2, "n3")):
                    tg = cp.tile([C, 1], f32, name=f"{key}g_{i}")
                    nc.sync.dma_start(out=tg[:], in_=d_bng[i, j])
                    tb = cp.tile([C, 1], f32, name=f"{key}b_{i}")
                    nc.sync.dma_start(out=tb[:], in_=d_bnb[i, j])
                    W[(key + "g", i)] = tg
                    W[(key + "b", i)] = tb
            hw1 = cp.tile([C, C // 2], f32r, name="hw1")
            nc.sync.dma_start(out=hw1[:], in_=d_hw1[:])
            hw2 = cp.tile([C // 2, C // 4], f32r, name="hw2")
            nc.sync.dma_start(out=hw2[:], in_=d_hw2[:])
            hw3 = cp.tile([C // 4, 1], f32r, name="hw3")
            nc.sync.dma_start(out=hw3[:], in_=d_hw3[:])
            hb1 = cp.tile([C // 2, 1], f32, name="hb1")
            nc.sync.dma_start(out=hb1[:], in_=d_hb1[:])
            hb2 = cp.tile([C // 4, 1], f32, name="hb2")
            nc.sync.dma_start(out=hb2[:], in_=d_hb2[:])
            hb3 = cp.tile([1, 1], f32, name="hb3")
            nc.sync.dma_start(out=hb3[:], in_=d_hb3[:])

            # ---------------- persistent feature tiles ----------------
            xT = bp.tile([C, N], f32r, name="xT")          # full transposed features
            nc.sync.dma_start(out=xT[:], in_=d_xT[:])
            xq = bp.tile([C, NO], f32r, name="xq")         # own columns
            nc.sync.dma_start(out=xq[:], in_=d_xq[:])
            x_nm = bp.tile([C, KT, C], f32r, name="x_nm")  # node-major x, 32 tiles
            v_aug = wp.tile([C, KT, 33 * HEADS], f32r, name="v_aug", tag="v_aug")

            def bn_params(s_sum, s_sq, g_ap, b_ap, pool, nm):
                """From global [128,1] sum/sumsq -> (s, t) with BN(u) = s*u + t."""
                mean = pool.tile([C, 1], f32, name=f"mean_{nm}", tag=f"bnp_{nm}0")
                nc.vector.tensor_scalar(out=mean[:], in0=s_sum, scalar1=1.0 / N,
                                        scalar2=None, op0=OP.mult)
                var = pool.tile([C, 1], f32, name=f"var_{nm}", tag=f"bnp_{nm}1")
                nc.vector.tensor_scalar(out=var[:], in0=s_sq, scalar1=1.0 / N,
                                        scalar2=None, op0=OP.mult)
                msq = pool.tile([C, 1], f32, name=f"msq_{nm}", tag=f"bnp_{nm}2")
                nc.vector.tensor_tensor(out=msq[:], in0=mean[:], in1=mean[:], op=OP.mult)
                nc.vector.tensor_tensor(out=var[:], in0=var[:], in1=msq[:], op=OP.subtract)
                lnv = pool.tile([C, 1], f32, name=f"lnv_{nm}", tag=f"bnp_{nm}3")
                nc.scalar.activation(lnv[:], var[:], AF.Ln, bias=epsc[:], scale=1.0)
                rst = pool.tile([C, 1], f32, name=f"rst_{nm}", tag=f"bnp_{nm}4")
                nc.scalar.activation(rst[:], lnv[:], AF.Exp, bias=0.0, scale=-0.5)
                s_ = pool.tile([C, 1], f32, name=f"s_{nm}", tag=f"bnp_{nm}5")
                nc.vector.tensor_tensor(out=s_[:], in0=rst[:], in1=g_ap, op=OP.mult)
                sm = pool.tile([C, 1], f32, name=f"sm_{nm}", tag=f"bnp_{nm}6")
                nc.vector.tensor_tensor(out=sm[:], in0=s_[:], in1=mean[:], op=OP.mult)
                t_ = pool.tile([C, 1], f32, name=f"t_{nm}", tag=f"bnp_{nm}7")
                nc.vector.tensor_tensor(out=t_[:], in0=b_ap, in1=sm[:], op=OP.subtract)
                return s_, t_

            def stats_into(u_ap, dst2_ap, nm):
                """Write [rowsum, rowsumsq] of u [128, 512] into dst2 [128, 2]."""
                sq = wp.tile([C, NO], f32, name=f"sq_{nm}", tag="sq_scratch")
                nc.vector.tensor_tensor(out=sq[:], in0=u_ap, in1=u_ap, op=OP.mult)
                with nc.allow_low_precision(reason="bn partial sums in f32r payload"):
                    nc.vector.reduce_sum(dst2_ap[:, 0:1], u_ap, axis=AX.X)
                    nc.vector.reduce_sum(dst2_ap[:, 1:2], sq[:], axis=AX.X)

            for i in range(L):
                # ======== transposes: x node-major (for GIN agg lhsT) ========
                for j in range(KT):
                    tp = pmm.tile([C, C], f32r, name=f"tp_{i}_{j}", tag="mm")
                    nc.tensor.transpose(tp[:], xT[:, j * C:(j + 1) * C], ident[:])
                    nc.vector.tensor_copy(out=x_nm[:, j, :], in_=tp[:])

                # ======== GIN branch ========
                agg = pagg.tile([C, NO], f32, name=f"agg_{i}", tag="agg")
                for j in range(KT):
                    at_t = sp.tile([C, NO], f32r, name=f"at_{i}_{j}", tag="at_stream")
                    nc.sync.dma_start(out=at_t[:], in_=d_AT[j * C:(j + 1) * C, :])
                    nc.tensor.matmul(agg[:], x_nm[:, j, :], at_t[:],
                                     start=(j == 0), stop=(j == KT - 1))
                z = wp.tile([C, NO], f32r, name=f"z_{i}", tag="z")
                nc.vector.tensor_tensor(out=z[:], in0=agg[:], in1=xq[:], op=OP.add)
                g1p = pmm.tile([C, NO], f32, name=f"g1p_{i}", tag="mm")
                nc.tensor.matmul(g1p[:], W[("g1T", i)][:], z[:], start=True, stop=True)
                r1 = wp.tile([C, NO], f32r, name=f"r1_{i}", tag="r1")
                nc.scalar.activation(r1[:], g1p[:], AF.Relu, bias=W[("g1b", i)][:], scale=1.0)
                g2p = pmm.tile([C, NO], f32, name=f"g2p_{i}", tag="mm")
                nc.tensor.matmul(g2p[:], W[("g2T", i)][:], r1[:], start=True, stop=True)
                u1 = wp.tile([C, NO], f32, name=f"u1_{i}", tag="u1")
                nc.vector.tensor_scalar(out=u1[:], in0=g2p[:], scalar1=W[("g2b", i)][:],
                                        scalar2=None, op0=OP.add)
                nc.vector.tensor_tensor(out=u1[:], in0=u1[:], in1=xq[:], op=OP.add)
                stats = wp.tile([C, 4], f32, name=f"stats_{i}", tag="stats")
                stats_into(u1[:], stats[:, 0:2], f"u1_{i}")

                # ======== attention: q/k/v projections ========
                qp = pmm.tile([C, NO], f32, name=f"qp_{i}", tag="mm")
                nc.tensor.matmul(qp[:], W[("wiT", i)][:, 0:C], xq[:], start=True, stop=True)
                qTs = wp.tile([C, NO], f32r, name=f"qTs_{i}", tag="qTs")
                nc.vector.tensor_scalar(out=qTs[:], in0=qp[:], scalar1=W[("wib", i)][:, 0:1],
                                        scalar2=None, op0=OP.add)
                qp3 = pmm.tile([D, NO], f32, name=f"qp3_{i}", tag="mm")
                nc.tensor.matmul(qp3[:], W[("wiT", i)][:, 3 * C + 32:3 * C + 64], xq[:],
                                 start=True, stop=True)
                qTs3 = wp.tile([D, NO], f32r, name=f"qTs3_{i}", tag="qTs3")
                nc.vector.tensor_scalar(out=qTs3[:], in0=qp3[:], scalar1=W[("wib", i)][0:D, 4:5],
                                        scalar2=None, op0=OP.add)
                kTs3 = wp.tile([D, N], f32r, name=f"kTs3_{i}", tag="kTs3")
                kTs = wp.tile([C, N], f32r, name=f"kTs_{i}", tag="kTs")
                vTs = wp.tile([C, N], f32r, name=f"vTs_{i}", tag="vTs")
                for cch in range(NQC):
                    csl = slice(cch * 512, (cch + 1) * 512)
                    kp = pmm.tile([C, 512], f32, name=f"kp_{i}_{cch}", tag="mm")
                    nc.tensor.matmul(kp[:], W[("wiT", i)][:, C:2 * C], xT[:, csl],
                                     start=True, stop=True)
                    nc.vector.tensor_scalar(out=kTs[:, csl], in0=kp[:],
                                            scalar1=W[("wib", i)][:, 1:2],
                                            scalar2=None, op0=OP.add)
                    vp = pmm.tile([C, 512], f32, name=f"vp_{i}_{cch}", tag="mm")
                    nc.tensor.matmul(vp[:], W[("wiT", i)][:, 2 * C:3 * C], xT[:, csl],
                                     start=True, stop=True)
                    nc.vector.tensor_scalar(out=vTs[:, csl], in0=vp[:],
                                            scalar1=W[("wib", i)][:, 2:3],
                                            scalar2=None, op0=OP.add)
                    kp3 = pmm.tile([D, 512], f32, name=f"kp3_{i}_{cch}", tag="mm")
                    nc.tensor.matmul(kp3[:], W[("wiT", i)][:, 3 * C:3 * C + 32], xT[:, csl],
                                     start=True, stop=True)
                    nc.vector.tensor_scalar(out=kTs3[:, csl], in0=kp3[:],
                                            scalar1=W[("wib", i)][0:D, 3:4],
                                            scalar2=None, op0=OP.add)
                # v -> node-major into v_aug (ones column last per head)
                nc.vector.tensor_copy(
                    out=v_aug[:, :, :].rearrange("p kt (h c) -> p kt h c", h=HEADS)[:, :, :, 32:33],
                    in_=onesc[:, 0:1].to_broadcast([C, KT, HEADS, 1]))
                for kt in range(KT):
                    vt = pmm.tile([C, C], f32r, name=f"vt_{i}_{kt}", tag="mm")
                    nc.tensor.transpose(vt[:], vTs[:, kt * C:(kt + 1) * C], ident[:])
                    for h in range(HEADS):
                        nc.vector.tensor_copy(
                            out=v_aug[:, kt, 33 * h:33 * h + 32],
                            in_=vt[:, h * D:(h + 1) * D])

                # ======== attention core: per head, per k-tile ========
                o_norm = []
                for h in range(HEADS):
                    hsl = slice(h * D, (h + 1) * D)
                    ot = pot.tile([33, 512], f32, name=f"ot_{i}_{h}", tag="ot")
                    for kt in range(KT):
                        st = pst.tile([C, NO], f32, name=f"st_{i}_{h}_{kt}", tag="st")
                        if h < 3:
                            nc.tensor.matmul(st[:], kTs[hsl, kt * C:(kt + 1) * C],
                                             qTs[hsl, :], start=True, stop=True)
                        else:
                            nc.tensor.matmul(st[:], kTs3[:, kt * C:(kt + 1) * C],
                                             qTs3[:, :], start=True, stop=True)
                        est = sp.tile([C, NO], f32r, name=f"est_{i}_{h}_{kt}", tag="est")
                        nc.scalar.activation(est[:], st[:], AF.Exp)
                        nc.tensor.matmul(ot[:], v_aug[:, kt, 33 * h:33 * (h + 1)],
                                         est[:], start=(kt == 0), stop=(kt == KT - 1))
                    # normalize: rows 0-31 / row 32
                    recs = wp.tile([33, 512], f32r, name=f"recs_{i}_{h}", tag="recs", bufs=2)
                    with nc.allow_low_precision(reason="softmax denom reciprocal in f32r"):
                        nc.vector.reciprocal(recs[32:33, :], ot[32:33, :])
                    rbp = pmm.tile([32, 512], f32, name=f"rbp_{i}_{h}", tag="mm")
                    nc.tensor.matmul(rbp[:], onesc[32:33, 0:32], recs[32:33, :],
                                     start=True, stop=True)
                    rbs = wp.tile([32, 512], f32, name=f"rbs_{i}_{h}", tag="rbs", bufs=2)
                    nc.vector.tensor_copy(out=rbs[:], in_=rbp[:])
                    on = wp.tile([32, 512], f32r, name=f"on_{i}_{h}", tag=f"on{h}")
                    nc.vector.tensor_tensor(out=on[:], in0=ot[0:32, :], in1=rbs[:], op=OP.mult)
                    o_norm.append(on)
                # out-projection: sum over heads, K=32 each
                ap_ = pmm.tile([C, NO], f32, name=f"ap_{i}", tag="mm")
                for h in range(HEADS):
                    nc.tensor.matmul(ap_[:], W[("woTh", i)][:, h * C:(h + 1) * C],
                                     o_norm[h][:], start=(h == 0), stop=(h == HEADS - 1))
                u2 = wp.tile([C, NO], f32, name=f"u2_{i}", tag="u2")
                nc.vector.tensor_scalar(out=u2[:], in0=ap_[:], scalar1=W[("wob", i)][:],
                                        scalar2=None, op0=OP.add)
                nc.vector.tensor_tensor(out=u2[:], in0=u2[:], in1=xq[:], op=OP.add)
                stats_into(u2[:], stats[:, 2:4], f"u2_{i}")

                # ======== AllReduce BN1+BN2 stats ========
                ar_in = dp.tile([C, 4], f32, name=f"ar_in_{i}", tag="ar_in")
                nc.sync.dma_start(out=ar_in[:], in_=stats[:])
                ar_out = dp.tile([C, 4], f32, name=f"ar_out_{i}", tag="ar_out",
                                 addr_space="Shared")
                nc.gpsimd.collective_compute("AllReduce", OP.add, replica_groups=RG,
                                             ins=[ar_in[:].opt()], outs=[ar_out[:].opt()])
                ar_sb = wp.tile([C, 4], f32, name=f"ar_sb_{i}", tag="ar_sb")
                nc.sync.dma_start(out=ar_sb[:], in_=ar_out[:])
                s1, t1 = bn_params(ar_sb[:, 0:1], ar_sb[:, 1:2],
                                   W[("n1g", i)][:], W[("n1b", i)][:], wp, f"bn1_{i}")
                s2, t2 = bn_params(ar_sb[:, 2:3], ar_sb[:, 3:4],
                                   W[("n2g", i)][:], W[("n2b", i)][:], wp, f"bn2_{i}")

                # ======== h = BN1(u1) + BN2(u2); MLP; yT ========
                ha = wp.tile([C, NO], f32, name=f"ha_{i}", tag="ha")
                nc.vector.tensor_scalar(out=ha[:], in0=u1[:], scalar1=s1[:],
                                        scalar2=t1[:], op0=OP.mult, op1=OP.add)
                hb = wp.tile([C, NO], f32, name=f"hb_{i}", tag="hbt")
                nc.vector.tensor_scalar(out=hb[:], in0=u2[:], scalar1=s2[:],
                                        scalar2=t2[:], op0=OP.mult, op1=OP.add)
                hh = wp.tile([C, NO], f32r, name=f"hh_{i}", tag="hh")
                nc.vector.tensor_tensor(out=hh[:], in0=ha[:], in1=hb[:], op=OP.add)
                ra = wp.tile([C, NO], f32r, name=f"ra_{i}", tag="ra")
                rb_ = wp.tile([C, NO], f32r, name=f"rb_{i}", tag="rbt")
                m1p = pmm.tile([C, NO], f32, name=f"m1pa_{i}", tag="mm")
                nc.tensor.matmul(m1p[:], W[("m1T", i)][:, 0:C], hh[:], start=True, stop=True)
                nc.scalar.activation(ra[:], m1p[:], AF.Relu, bias=W[("m1b", i)][:, 0:1], scale=1.0)
                m1p2 = pmm.tile([C, NO], f32, name=f"m1pb_{i}", tag="mm")
                nc.tensor.matmul(m1p2[:], W[("m1T", i)][:, C:2 * C], hh[:], start=True, stop=True)
                nc.scalar.activation(rb_[:], m1p2[:], AF.Relu, bias=W[("m1b", i)][:, 1:2], scale=1.0)
                m2p = pmm.tile([C, NO], f32, name=f"m2p_{i}", tag="mm")
                nc.tensor.matmul(m2p[:], W[("m2T", i)][:, 0, :], ra[:], start=True, stop=False)
                nc.tensor.matmul(m2p[:], W[("m2T", i)][:, 1, :], rb_[:], start=False, stop=True)
                payload = wp.tile([C, NO + 4], f32r, name=f"payload_{i}", tag="payload")
                yT = payload[:, 0:NO]
                nc.vector.tensor_scalar(out=yT, in0=m2p[:], scalar1=W[("m2b", i)][:],
                                        scalar2=None, op0=OP.add)
                nc.vector.tensor_tensor(out=yT, in0=yT, in1=hh[:], op=OP.add)

                if i < L - 1:
                    # ======== AllGather yT + BN3 stats ========
                    stats_into(yT, payload[:, NO:NO + 2], f"y_{i}")
                    ag_in = dp.tile([C, NO + 4], f32r, name=f"ag_in_{i}", tag="ag_in")
                    nc.sync.dma_start(out=ag_in[:], in_=payload[:])
                    ag_out = dp.tile([NCORES, C, NO + 4], f32r, name=f"ag_out_{i}",
                                     tag="ag_out", addr_space="Shared")
                    nc.gpsimd.collective_compute("AllGather", OP.bypass, replica_groups=RG,
                                                 ins=[ag_in[:].opt()], outs=[ag_out[:].opt()])
                    # gathered raw yT -> xT (one DMA, rank-major -> column blocks)
                    nc.sync.dma_start(
                        out=xT[:].rearrange("p (r c) -> p r c", r=NCORES),
                        in_=ag_out[:, :, 0:NO].rearrange("r p c -> p r c"))
                    st3 = wp.tile([C, NCORES, 2], f32r, name=f"st3_{i}", tag="st3")
                    nc.sync.dma_start(out=st3[:],
                                      in_=ag_out[:, :, NO:NO + 2].rearrange("r p c -> p r c"))
                    st3g = wp.tile([C, 2], f32, name=f"st3g_{i}", tag="st3g")
                    nc.vector.reduce_sum(st3g[:],
                                         st3[:].rearrange("p r c -> p c r"), axis=AX.X)
                    s3, t3 = bn_params(st3g[:, 0:1], st3g[:, 1:2],
                                       W[("n3g", i)][:], W[("n3b", i)][:], wp, f"bn3_{i}")
                    # BN3 applied in place on gathered xT and locally on own yT
                    nc.vector.tensor_scalar(out=xT[:], in0=xT[:], scalar1=s3[:],
                                            scalar2=t3[:], op0=OP.mult, op1=OP.add)
                    nc.vector.tensor_scalar(out=xq[:], in0=yT, scalar1=s3[:],
                                            scalar2=t3[:], op0=OP.mult, op1=OP.add)
                else:
                    # last layer: AR of BN3 stats only, then head MLP on own columns
                    st3 = wp.tile([C, 2], f32, name=f"st3_{i}", tag="st3f")
                    stats_into(yT, st3[:], f"y_{i}")
                    ar3_in = dp.tile([C, 2], f32, name="ar3_in", tag="ar3_in")
                    nc.sync.dma_start(out=ar3_in[:], in_=st3[:])
                    ar3_out = dp.tile([C, 2], f32, name="ar3_out", tag="ar3_out",
                                      addr_space="Shared")
                    nc.gpsimd.collective_compute("AllReduce", OP.add, replica_groups=RG,
                                                 ins=[ar3_in[:].opt()], outs=[ar3_out[:].opt()])
                    ar3_sb = wp.tile([C, 2], f32, name="ar3_sb", tag="ar3_sb")
                    nc.sync.dma_start(out=ar3_sb[:], in_=ar3_out[:])
                    s3, t3 = bn_params(ar3_sb[:, 0:1], ar3_sb[:, 1:2],
                                       W[("n3g", i)][:], W[("n3b", i)][:], wp, f"bn3_{i}")
                    xf = wp.tile([C, NO], f32r, name="xf", tag="xf")
                    nc.vector.tensor_scalar(out=xf[:], in0=yT, scalar1=s3[:],
                                            scalar2=t3[:], op0=OP.mult, op1=OP.add)
                    h1p = pmm.tile([C // 2, NO], f32, name="h1p", tag="mm")
                    nc.tensor.matmul(h1p[:], hw1[:], xf[:], start=True, stop=True)
                    hr1 = wp.tile([C // 2, NO], f32r, name="hr1", tag="hr1")
                    nc.scalar.activation(hr1[:], h1p[:], AF.Relu, bias=hb1[:], scale=1.0)
                    h2p = pmm.tile([C // 4, NO], f32, name="h2p", tag="mm")
                    nc.tensor.matmul(h2p[:], hw2[:], hr1[:], start=True, stop=True)
                    hr2 = wp.tile([C // 4, NO], f32r, name="hr2", tag="hr2")
                    nc.scalar.activation(hr2[:], h2p[:], AF.Relu, bias=hb2[:], scale=1.0)
                    h3p = pmm.tile([1, NO], f32, name="h3p", tag="mm")
                    nc.tensor.matmul(h3p[:], hw3[:], hr2[:], start=True, stop=True)
                    outs = wp.tile([1, NO], f32, name="outs", tag="outs")
                    nc.vector.tensor_scalar(out=outs[:], in0=h3p[:], scalar1=hb3[:],
                                            scalar2=None, op0=OP.add)
                    nc.sync.dma_start(out=d_out[:], in_=outs[:])

    nc.compile()
    return nc


def _host_prep(inputs):
    x = np.asarray(inputs["x"], dtype=np.float32)
    ei = np.asarray(inputs["edge_index"])
    src, dst = np.asarray(ei[0], dtype=np.int64), np.asarray(ei[1], dtype=np.int64)
    AT = np.zeros((N, N), dtype=np.float32)
    np.add.at(AT, (src, dst), 1.0)

    xT = np.ascontiguousarray(x.T)
    sd = 1.0 / np.sqrt(np.float32(D))

    common = {
        "xT": xT,
        "ident": np.eye(C, dtype=np.float32),
        "onesc": np.ones((C, C), dtype=np.float32),
        "hw1T": np.ascontiguousarray(np.asarray(inputs["head_w1"], np.float32).T),
        "hw2T": np.ascontiguousarray(np.asarray(inputs["head_w2"], np.float32).T),
        "hw3T": np.ascontiguousarray(np.asarray(inputs["head_w3"], np.float32).T),
        "hb1": np.asarray(inputs["head_b1"], np.float32).reshape(-1, 1),
        "hb2": np.asarray(inputs["head_b2"], np.float32).reshape(-1, 1),
        "hb3": np.asarray(inputs["head_b3"], np.float32).reshape(-1, 1),
    }
    wiT = np.stack([np.asarray(inputs["attn_in_w"][i], np.float32).T for i in range(L)])
    wib = np.stack([np.ascontiguousarray(
        np.asarray(inputs["attn_in_b"][i], np.float32).reshape(3, C).T)
        for i in range(L)])
    wiT = wiT.copy()
    wib = wib.copy()
    wiT[:, :, 0:C] *= sd          # fold 1/sqrt(d) into q projection
    wib[:, :, 0] *= sd
    # head-3 q/k duplicates at partition base 0 (PE quadrant-3 workaround)
    wiT = np.concatenate([wiT,
                          wiT[:, :, C + 3 * D:C + 4 * D],        # k head3
                          wiT[:, :, 3 * D:4 * D]], axis=2)        # q head3 (scaled)
    wib3 = np.zeros((L, C, 2), dtype=np.float32)
    wib3[:, 0:D, 0] = wib[:, 3 * D:4 * D, 1]                      # k head3 bias
    wib3[:, 0:D, 1] = wib[:, 3 * D:4 * D, 0]                      # q head3 bias (scaled)
    wib = np.concatenate([wib, wib3], axis=2)
    woTh = np.zeros((L, D, 4 * C), dtype=np.float32)
    for i in range(L):
        woT = np.asarray(inputs["attn_out_w"][i], np.float32).T   # [C, C]
        for h in range(HEADS):
            woTh[i, :, h * C:(h + 1) * C] = woT[h * D:(h + 1) * D, :]
    common.update({
        "wiT": wiT, "wib": wib, "woTh": woTh,
        "wob": np.stack([np.asarray(inputs["attn_out_b"][i], np.float32).reshape(-1, 1)
                         for i in range(L)]),
        "g1T": np.stack([np.asarray(inputs["gin_w1"][i], np.float32).T for i in range(L)]),
        "g1b": np.stack([np.asarray(inputs["gin_b1"][i], np.float32).reshape(-1, 1)
                         for i in range(L)]),
        "g2T": np.stack([np.asarray(inputs["gin_w2"][i], np.float32).T for i in range(L)]),
        "g2b": np.stack([np.asarray(inputs["gin_b2"][i], np.float32).reshape(-1, 1)
                         for i in range(L)]),
        "m1T": np.stack([np.asarray(inputs["mlp_w1"][i], np.float32).T for i in range(L)]),
        "m1b": np.stack([np.ascontiguousarray(
            np.asarray(inputs["mlp_b1"][i], np.float32).reshape(2, C).T)
            for i in range(L)]),
        "m2T": np.stack([np.ascontiguousarray(
            np.asarray(inputs["mlp_w2"][i], np.float32).T.reshape(2, C, C).transpose(1, 0, 2))
            for i in range(L)]),
        "m2b": np.stack([np.asarray(inputs["mlp_b2"][i], np.float32).reshape(-1, 1)
                         for i in range(L)]),
        "bng": np.stack([np.stack([np.asarray(inputs[k][i], np.float32).reshape(-1, 1)
                                   for k in ("n1_g", "n2_g", "n3_g")]) for i in range(L)]),
        "bnb": np.stack([np.stack([np.asarray(inputs[k][i], np.float32).reshape(-1, 1)
                                   for k in ("n1_b", "n2_b", "n3_b")]) for i in range(L)]),
    })
    in_maps = []
    for r in range(NCORES):
        m = dict(common)
        m["xq"] = np.ascontiguousarray(xT[:, r * NO:(r + 1) * NO])
        m["AT"] = np.ascontiguousarray(AT[:, r * NO:(r + 1) * NO])
        in_maps.append(m)
    return in_maps


def kernel(**inputs):
    if "nc" not in _CACHED:
        _CACHED["nc"] = _build()
    nc = _CACHED["nc"]
    in_maps = _host_prep(inputs)
    res = bass_utils.run_bass_kernel_spmd(nc, in_maps, core_ids=list(range(NCORES)))
    y = np.zeros((N, 1), dtype=np.float32)
    for r in range(NCORES):
        y[r * NO:(r + 1) * NO, 0] = res.results[r]["out"][0]
    return y


# revision 11
# speedup vs baseline: 1.9312x; 1.9312x over previous
"""GPS (GIN + global attention) kernel for 8 TRN2 NeuronCores.

Sharding: nodes split 512/core. Features kept transposed (xT [C=128, nodes]).
- GIN aggregation = dense A.T slice matmul (A built host-side from edge_index,
  streamed per layer from HBM, f32r full-rate).
- Attention: scores transposed ST[k, q] per head (K=32 matmuls, N=512 free),
  exp on ACT (no max subtraction; scores are O(1)), denominator via ones
  column appended to V, AV accumulates oT[33, 512] over k-tiles.
- BatchNorm over nodes: per-core [sum, sumsq] + cross-core AllReduce;
  rsqrt computed as exp(-0.5*ln(var+eps)) (single ACT table set).
- Per layer: one AllReduce [128,4] (BN1+BN2 stats) and one AllGather of the
  pre-BN3 layer output with BN3 stats piggybacked; BN3 applied post-gather.
"""
import sys
sys.path.insert(0, "/opt/trn_rl_repo")

import numpy as np
import concourse.bacc as bacc
import concourse.mybir as mybir
import concourse.tile as tile
from concourse import bass_utils

N = 4096
C = 128
L = 4
E = 131072
HEADS = 4
D = C // HEADS            # 32
NCORES = 8
NO = N // NCORES          # 512 nodes per core
BN_EPS = 1e-5
RG = [[i for i in range(NCORES)]]

f32 = mybir.dt.float32
f32r = mybir.dt.float32r
AF = mybir.ActivationFunctionType
OP = mybir.AluOpType
AX = mybir.AxisListType

_CACHED = {}


def _build(depth_mult=1):
    nc = bacc.Bacc("TRN2", target_bir_lowering=False, num_devices=NCORES)

    # ---------------- DRAM I/O ----------------
    d_xT = nc.dram_tensor("xT", [C, N], f32r, kind="ExternalInput")
    d_xq = nc.dram_tensor("xq", [C, NO], f32r, kind="ExternalInput")       # per-core
    d_AT = nc.dram_tensor("AT", [N, NO], f32r, kind="ExternalInput")       # per-core
    d_ident = nc.dram_tensor("ident", [C, C], f32r, kind="ExternalInput")
    d_ones = nc.dram_tensor("onesc", [C, C], f32r, kind="ExternalInput")
    d_wiT = nc.dram_tensor("wiT", [L, C, 3 * C + 64], f32r, kind="ExternalInput")
    d_wib = nc.dram_tensor("wib", [L, C, 5], f32, kind="ExternalInput")
    d_woTh = nc.dram_tensor("woTh", [L, D, 4 * C], f32r, kind="ExternalInput")
    d_wob = nc.dram_tensor("wob", [L, C, 1], f32, kind="ExternalInput")
    d_g1T = nc.dram_tensor("g1T", [L, C, C], f32r, kind="ExternalInput")
    d_g1b = nc.dram_tensor("g1b", [L, C, 1], f32, kind="ExternalInput")
    d_g2T = nc.dram_tensor("g2T", [L, C, C], f32r, kind="ExternalInput")
    d_g2b = nc.dram_tensor("g2b", [L, C, 1], f32, kind="ExternalInput")
    d_m1T = nc.dram_tensor("m1T", [L, C, 2 * C], f32r, kind="ExternalInput")
    d_m1b = nc.dram_tensor("m1b", [L, C, 2], f32, kind="ExternalInput")
    d_m2T = nc.dram_tensor("m2T", [L, C, 2, C], f32r, kind="ExternalInput")
    d_m2b = nc.dram_tensor("m2b", [L, C, 1], f32, kind="ExternalInput")
    d_bng = nc.dram_tensor("bng", [L, 3, C, 1], f32, kind="ExternalInput")
    d_bnb = nc.dram_tensor("bnb", [L, 3, C, 1], f32, kind="ExternalInput")
    d_hw1 = nc.dram_tensor("hw1T", [C, C // 2], f32r, kind="ExternalInput")
    d_hw2 = nc.dram_tensor("hw2T", [C // 2, C // 4], f32r, kind="ExternalInput")
    d_hw3 = nc.dram_tensor("hw3T", [C // 4, 1], f32r, kind="ExternalInput")
    d_hb1 = nc.dram_tensor("hb1", [C // 2, 1], f32, kind="ExternalInput")
    d_hb2 = nc.dram_tensor("hb2", [C // 4, 1], f32, kind="ExternalInput")
    d_hb3 = nc.dram_tensor("hb3", [1, 1], f32, kind="ExternalInput")
    d_out = nc.dram_tensor("out", [1, NO], f32, kind="ExternalOutput")

    KT = N // C               # 32 k-tiles of 128 nodes
    NQC = N // 512            # 8 column chunks of 512

    with tile.TileContext(nc) as tc:
        with tc.tile_pool(name="const", bufs=1) as cp, \
             tc.tile_pool(name="big", bufs=1) as bp, \
             tc.tile_pool(name="work", bufs=1) as wp, \
             tc.tile_pool(name="stream", bufs=4) as sp, \
             tc.tile_pool(name="psum_st", bufs=3, space="PSUM") as pst, \
             tc.tile_pool(name="psum_ot", bufs=2, space="PSUM") as pot, \
             tc.tile_pool(name="psum_agg", bufs=1, space="PSUM") as pagg, \
             tc.tile_pool(name="psum_mm", bufs=2, space="PSUM") as pmm, \
             tc.tile_pool(name="dram", bufs=2, space="DRAM") as dp:

            # ---------------- constants ----------------
            ident = cp.tile([C, C], f32r, name="ident")
            nc.sync.dma_start(out=ident[:], in_=d_ident[:])
            onesc = cp.tile([C, C], f32r, name="onesc")
            nc.sync.dma_start(out=onesc[:], in_=d_ones[:])
            epsc = cp.tile([C, 1], f32, name="epsc")
            nc.vector.memset(epsc[:], BN_EPS)

            W = {}
            for i in range(L):
                for key, dt_, dram, shp in (
                    ("wiT", f32r, d_wiT, [C, 3 * C + 64]),
                    ("wib", f32, d_wib, [C, 5]),
                    ("woTh", f32r, d_woTh, [D, 4 * C]),
                    ("wob", f32, d_wob, [C, 1]),
                    ("g1T", f32r, d_g1T, [C, C]),
                    ("g1b", f32, d_g1b, [C, 1]),
                    ("g2T", f32r, d_g2T, [C, C]),
                    ("g2b", f32, d_g2b, [C, 1]),
                    ("m1T", f32r, d_m1T, [C, 2 * C]),
                    ("m1b", f32, d_m1b, [C, 2]),
                    ("m2T", f32r, d_m2T, [C, 2, C]),
                    ("m2b", f32, d_m2b, [C, 1]),
                ):
                    t = cp.tile(shp, dt_, name=f"{key}_{i}")
                    nc.sync.dma_start(out=t[:], in_=dram[i])
                    W[(key, i)] = t
                for j, key in ((0, "n1"), (1, "n2"), (2, "n3")):
                    tg = cp.tile([C, 1], f32, name=f"{key}g_{i}")
                    nc.sync.dma_start(out=tg[:], in_=d_bng[i, j])
                    tb = cp.tile([C, 1], f32, name=f"{key}b_{i}")
                    nc.sync.dma_start(out=tb[:], in_=d_bnb[i, j])
                    W[(key + "g", i)] = tg
                    W[(key + "b", i)] = tb
            hw1 = cp.tile([C, C // 2], f32r, name="hw1")
            nc.sync.dma_start(out=hw1[:], in_=d_hw1[:])
            hw2 = cp.tile([C // 2, C // 4], f32r, name="hw2")
            nc.sync.dma_start(out=hw2[:], in_=d_hw2[:])
            hw3 = cp.tile([C // 4, 1], f32r, name="hw3")
            nc.sync.dma_start(out=hw3[:], in_=d_hw3[:])
            hb1 = cp.tile([C // 2, 1], f32, name="hb1")
            nc.sync.dma_start(out=hb1[:], in_=d_hb1[:])
            hb2 = cp.tile([C // 4, 1], f32, name="hb2")
            nc.sync.dma_start(out=hb2[:], in_=d_hb2[:])
            hb3 = cp.tile([1, 1], f32, name="hb3")
            nc.sync.dma_start(out=hb3[:], in_=d_hb3[:])

            # ---------------- persistent feature tiles ----------------
            xT = bp.tile([C, N], f32r, name="xT")          # full transposed features
            nc.sync.dma_start(out=xT[:], in_=d_xT[:])
            xq = bp.tile([C, NO], f32r, name="xq")         # own columns
            nc.sync.dma_start(out=xq[:], in_=d_xq[:])
            x_nm = bp.tile([C, KT, C], f32r, name="x_nm")  # node-major x, 32 tiles
            v_aug = wp.tile([C, KT, 33 * HEADS], f32r, name="v_aug", tag="v_aug")

            def bn_params(s_sum, s_sq, g_ap, b_ap, pool, nm):
                """From global [128,1] sum/sumsq -> (s, t) with BN(u) = s*u + t."""
                mean = pool.tile([C, 1], f32, name=f"mean_{nm}", tag=f"bnp_{nm}0")
                nc.vector.tensor_scalar(out=mean[:], in0=s_sum, scalar1=1.0 / N,
                                        scalar2=None, op0=OP.mult)
                var = pool.tile([C, 1], f32, name=f"var_{nm}", tag=f"bnp_{nm}1")
                nc.vector.tensor_scalar(out=var[:], in0=s_sq, scalar1=1.0 / N,
                                        scalar2=None, op0=OP.mult)
                msq = pool.tile([C, 1], f32, name=f"msq_{nm}", tag=f"bnp_{nm}2")
                nc.vector.tensor_tensor(out=msq[:], in0=mean[:], in1=mean[:], op=OP.mult)
                nc.vector.tensor_tensor(out=var[:], in0=var[:], in1=msq[:], op=OP.subtract)
                lnv = pool.tile([C, 1], f32, name=f"lnv_{nm}", tag=f"bnp_{nm}3")
                nc.scalar.activation(lnv[:], var[:], AF.Ln, bias=epsc[:], scale=1.0)
                rst = pool.tile([C, 1], f32, name=f"rst_{nm}", tag=f"bnp_{nm}4")
                nc.scalar.activation(rst[:], lnv[:], AF.Exp, bias=0.0, scale=-0.5)
                s_ = pool.tile([C, 1], f32, name=f"s_{nm}", tag=f"bnp_{nm}5")
                nc.vector.tensor_tensor(out=s_[:], in0=rst[:], in1=g_ap, op=OP.mult)
                sm = pool.tile([C, 1], f32, name=f"sm_{nm}", tag=f"bnp_{nm}6")
                nc.vector.tensor_tensor(out=sm[:], in0=s_[:], in1=mean[:], op=OP.mult)
                t_ = pool.tile([C, 1], f32, name=f"t_{nm}", tag=f"bnp_{nm}7")
                nc.vector.tensor_tensor(out=t_[:], in0=b_ap, in1=sm[:], op=OP.subtract)
                return s_, t_

            def stats_into(u_ap, dst2_ap, nm):
                """Write [rowsum, rowsumsq] of u [128, 512] into dst2 [128, 2]."""
                sq = wp.tile([C, NO], f32, name=f"sq_{nm}", tag="sq_scratch")
                nc.vector.tensor_tensor(out=sq[:], in0=u_ap, in1=u_ap, op=OP.mult)
                with nc.allow_low_precision(reason="bn partial sums in f32r payload"):
                    nc.vector.reduce_sum(dst2_ap[:, 0:1], u_ap, axis=AX.X)
                    nc.vector.reduce_sum(dst2_ap[:, 1:2], sq[:], axis=AX.X)

            for li in range(L * depth_mult):
                i = li % L
                is_last = li == L * depth_mult - 1
                # ======== transposes: x node-major (for GIN agg lhsT) ========
                for j in range(KT):
                    tp = pmm.tile([C, C], f32r, name=f"tp_{li}_{j}", tag="mm")
                    nc.tensor.transpose(tp[:], xT[:, j * C:(j + 1) * C], ident[:])
                    nc.vector.tensor_copy(out=x_nm[:, j, :], in_=tp[:])

                # ======== GIN branch ========
                agg = pagg.tile([C, NO], f32, name=f"agg_{li}", tag="agg")
                for j in range(KT):
                    at_t = sp.tile([C, NO], f32r, name=f"at_{li}_{j}", tag="at_stream")
                    nc.sync.dma_start(out=at_t[:], in_=d_AT[j * C:(j + 1) * C, :])
                    nc.tensor.matmul(agg[:], x_nm[:, j, :], at_t[:],
                                     start=(j == 0), stop=(j == KT - 1))
                z = wp.tile([C, NO], f32r, name=f"z_{li}", tag="z")
                nc.vector.tensor_tensor(out=z[:], in0=agg[:], in1=xq[:], op=OP.add)
                g1p = pmm.tile([C, NO], f32, name=f"g1p_{li}", tag="mm")
                nc.tensor.matmul(g1p[:], W[("g1T", i)][:], z[:], start=True, stop=True)
                r1 = wp.tile([C, NO], f32r, name=f"r1_{li}", tag="r1")
                nc.scalar.activation(r1[:], g1p[:], AF.Relu, bias=W[("g1b", i)][:], scale=1.0)
                g2p = pmm.tile([C, NO], f32, name=f"g2p_{li}", tag="mm")
                nc.tensor.matmul(g2p[:], W[("g2T", i)][:], r1[:], start=True, stop=True)
                u1 = wp.tile([C, NO], f32, name=f"u1_{li}", tag="u1")
                nc.vector.tensor_scalar(out=u1[:], in0=g2p[:], scalar1=W[("g2b", i)][:],
                                        scalar2=None, op0=OP.add)
                nc.vector.tensor_tensor(out=u1[:], in0=u1[:], in1=xq[:], op=OP.add)
                stats = wp.tile([C, 4], f32, name=f"stats_{li}", tag="stats")
                stats_into(u1[:], stats[:, 0:2], f"u1_{li}")

                # ======== attention: q/k/v projections ========
                qp = pmm.tile([C, NO], f32, name=f"qp_{li}", tag="mm")
                nc.tensor.matmul(qp[:], W[("wiT", i)][:, 0:C], xq[:], start=True, stop=True)
                qTs = wp.tile([C, NO], f32r, name=f"qTs_{li}", tag="qTs")
                nc.vector.tensor_scalar(out=qTs[:], in0=qp[:], scalar1=W[("wib", i)][:, 0:1],
                                        scalar2=None, op0=OP.add)
                qp3 = pmm.tile([D, NO], f32, name=f"qp3_{li}", tag="mm")
                nc.tensor.matmul(qp3[:], W[("wiT", i)][:, 3 * C + 32:3 * C + 64], xq[:],
                                 start=True, stop=True)
                qTs3 = wp.tile([D, NO], f32r, name=f"qTs3_{li}", tag="qTs3")
                nc.vector.tensor_scalar(out=qTs3[:], in0=qp3[:], scalar1=W[("wib", i)][0:D, 4:5],
                                        scalar2=None, op0=OP.add)
                kTs3 = wp.tile([D, N], f32r, name=f"kTs3_{li}", tag="kTs3")
                kTs = wp.tile([C, N], f32r, name=f"kTs_{li}", tag="kTs")
                vTs = wp.tile([C, N], f32r, name=f"vTs_{li}", tag="vTs")
                for cch in range(NQC):
                    csl = slice(cch * 512, (cch + 1) * 512)
                    kp = pmm.tile([C, 512], f32, name=f"kp_{li}_{cch}", tag="mm")
                    nc.tensor.matmul(kp[:], W[("wiT", i)][:, C:2 * C], xT[:, csl],
                                     start=True, stop=True)
                    nc.vector.tensor_scalar(out=kTs[:, csl], in0=kp[:],
                                            scalar1=W[("wib", i)][:, 1:2],
                                            scalar2=None, op0=OP.add)
                    vp = pmm.tile([C, 512], f32, name=f"vp_{li}_{cch}", tag="mm")
                    nc.tensor.matmul(vp[:], W[("wiT", i)][:, 2 * C:3 * C], xT[:, csl],
                                     start=True, stop=True)
                    nc.vector.tensor_scalar(out=vTs[:, csl], in0=vp[:],
                                            scalar1=W[("wib", i)][:, 2:3],
                                            scalar2=None, op0=OP.add)
                    kp3 = pmm.tile([D, 512], f32, name=f"kp3_{li}_{cch}", tag="mm")
                    nc.tensor.matmul(kp3[:], W[("wiT", i)][:, 3 * C:3 * C + 32], xT[:, csl],
                                     start=True, stop=True)
                    nc.vector.tensor_scalar(out=kTs3[:, csl], in0=kp3[:],
                                            scalar1=W[("wib", i)][0:D, 3:4],
                                            scalar2=None, op0=OP.add)
                # v -> node-major into v_aug (ones column last per head)
                nc.vector.tensor_copy(
                    out=v_aug[:, :, :].rearrange("p kt (h c) -> p kt h c", h=HEADS)[:, :, :, 32:33],
                    in_=onesc[:, 0:1].to_broadcast([C, KT, HEADS, 1]))
                for kt in range(KT):
                    vt = pmm.tile([C, C], f32r, name=f"vt_{li}_{kt}", tag="mm")
                    nc.tensor.transpose(vt[:], vTs[:, kt * C:(kt + 1) * C], ident[:])
                    for h in range(HEADS):
                        nc.vector.tensor_copy(
                            out=v_aug[:, kt, 33 * h:33 * h + 32],
                            in_=vt[:, h * D:(h + 1) * D])

                # ======== attention core: per head, per k-tile ========
                o_norm = []
                for h in range(HEADS):
                    hsl = slice(h * D, (h + 1) * D)
                    ot = pot.tile([33, 512], f32, name=f"ot_{li}_{h}", tag="ot")
                    for kt in range(KT):
                        st = pst.tile([C, NO], f32, name=f"st_{li}_{h}_{kt}", tag="st")
                        if h < 3:
                            nc.tensor.matmul(st[:], kTs[hsl, kt * C:(kt + 1) * C],
                                             qTs[hsl, :], start=True, stop=True)
                        else:
                            nc.tensor.matmul(st[:], kTs3[:, kt * C:(kt + 1) * C],
                                             qTs3[:, :], start=True, stop=True)
                        est = sp.tile([C, NO], f32r, name=f"est_{li}_{h}_{kt}", tag="est")
                        nc.scalar.activation(est[:], st[:], AF.Exp)
                        nc.tensor.matmul(ot[:], v_aug[:, kt, 33 * h:33 * (h + 1)],
                                         est[:], start=(kt == 0), stop=(kt == KT - 1))
                    # normalize: rows 0-31 / row 32
                    recs = wp.tile([33, 512], f32r, name=f"recs_{li}_{h}", tag="recs", bufs=2)
                    with nc.allow_low_precision(reason="softmax denom reciprocal in f32r"):
                        nc.vector.reciprocal(recs[32:33, :], ot[32:33, :])
                    rbp = pmm.tile([32, 512], f32, name=f"rbp_{li}_{h}", tag="mm")
                    nc.tensor.matmul(rbp[:], onesc[32:33, 0:32], recs[32:33, :],
                                     start=True, stop=True)
                    rbs = wp.tile([32, 512], f32, name=f"rbs_{li}_{h}", tag="rbs", bufs=2)
                    nc.vector.tensor_copy(out=rbs[:], in_=rbp[:])
                    on = wp.tile([32, 512], f32r, name=f"on_{li}_{h}", tag=f"on{h}")
                    nc.vector.tensor_tensor(out=on[:], in0=ot[0:32, :], in1=rbs[:], op=OP.mult)
                    o_norm.append(on)
                # out-projection: sum over heads, K=32 each
                ap_ = pmm.tile([C, NO], f32, name=f"ap_{li}", tag="mm")
                for h in range(HEADS):
                    nc.tensor.matmul(ap_[:], W[("woTh", i)][:, h * C:(h + 1) * C],
                                     o_norm[h][:], start=(h == 0), stop=(h == HEADS - 1))
                u2 = wp.tile([C, NO], f32, name=f"u2_{li}", tag="u2")
                nc.vector.tensor_scalar(out=u2[:], in0=ap_[:], scalar1=W[("wob", i)][:],
                                        scalar2=None, op0=OP.add)
                nc.vector.tensor_tensor(out=u2[:], in0=u2[:], in1=xq[:], op=OP.add)
                stats_into(u2[:], stats[:, 2:4], f"u2_{li}")

                # ======== AllReduce BN1+BN2 stats ========
                ar_in = dp.tile([C, 4], f32, name=f"ar_in_{li}", tag="ar_in")
                nc.sync.dma_start(out=ar_in[:], in_=stats[:])
                ar_out = dp.tile([C, 4], f32, name=f"ar_out_{li}", tag="ar_out",
                                 addr_space="Shared")
                nc.gpsimd.collective_compute("AllReduce", OP.add, replica_groups=RG,
                                             ins=[ar_in[:].opt()], outs=[ar_out[:].opt()])
                ar_sb = wp.tile([C, 4], f32, name=f"ar_sb_{li}", tag="ar_sb")
                nc.sync.dma_start(out=ar_sb[:], in_=ar_out[:])
                s1, t1 = bn_params(ar_sb[:, 0:1], ar_sb[:, 1:2],
                                   W[("n1g", i)][:], W[("n1b", i)][:], wp, f"bn1_{li}")
                s2, t2 = bn_params(ar_sb[:, 2:3], ar_sb[:, 3:4],
                                   W[("n2g", i)][:], W[("n2b", i)][:], wp, f"bn2_{li}")

                # ======== h = BN1(u1) + BN2(u2); MLP; yT ========
                ha = wp.tile([C, NO], f32, name=f"ha_{li}", tag="ha")
                nc.vector.tensor_scalar(out=ha[:], in0=u1[:], scalar1=s1[:],
                                        scalar2=t1[:], op0=OP.mult, op1=OP.add)
                hb = wp.tile([C, NO], f32, name=f"hb_{li}", tag="hbt")
                nc.vector.tensor_scalar(out=hb[:], in0=u2[:], scalar1=s2[:],
                                        scalar2=t2[:], op0=OP.mult, op1=OP.add)
                hh = wp.tile([C, NO], f32r, name=f"hh_{li}", tag="hh")
                nc.vector.tensor_tensor(out=hh[:], in0=ha[:], in1=hb[:], op=OP.add)
                ra = wp.tile([C, NO], f32r, name=f"ra_{li}", tag="ra")
                rb_ = wp.tile([C, NO], f32r, name=f"rb_{li}", tag="rbt")
                m1p = pmm.tile([C, NO], f32, name=f"m1pa_{li}", tag="mm")
                nc.tensor.matmul(m1p[:], W[("m1T", i)][:, 0:C], hh[:], start=True, stop=True)
                nc.scalar.activation(ra[:], m1p[:], AF.Relu, bias=W[("m1b", i)][:, 0:1], scale=1.0)
                m1p2 = pmm.tile([C, NO], f32, name=f"m1pb_{li}", tag="mm")
                nc.tensor.matmul(m1p2[:], W[("m1T", i)][:, C:2 * C], hh[:], start=True, stop=True)
                nc.scalar.activation(rb_[:], m1p2[:], AF.Relu, bias=W[("m1b", i)][:, 1:2], scale=1.0)
                m2p = pmm.tile([C, NO], f32, name=f"m2p_{li}", tag="mm")
                nc.tensor.matmul(m2p[:], W[("m2T", i)][:, 0, :], ra[:], start=True, stop=False)
                nc.tensor.matmul(m2p[:], W[("m2T", i)][:, 1, :], rb_[:], start=False, stop=True)
                payload = wp.tile([C, NO + 4], f32r, name=f"payload_{li}", tag="payload")
                yT = payload[:, 0:NO]
                nc.vector.tensor_scalar(out=yT, in0=m2p[:], scalar1=W[("m2b", i)][:],
                                        scalar2=None, op0=OP.add)
                nc.vector.tensor_tensor(out=yT, in0=yT, in1=hh[:], op=OP.add)

                if not is_last:
                    # ======== AllGather yT + BN3 stats ========
                    stats_into(yT, payload[:, NO:NO + 2], f"y_{li}")
                    ag_in = dp.tile([C, NO + 4], f32r, name=f"ag_in_{li}", tag="ag_in")
                    nc.sync.dma_start(out=ag_in[:], in_=payload[:])
                    ag_out = dp.tile([NCORES, C, NO + 4], f32r, name=f"ag_out_{li}",
                                     tag="ag_out", addr_space="Shared")
                    nc.gpsimd.collective_compute("AllGather", OP.bypass, replica_groups=RG,
                                                 ins=[ag_in[:].opt()], outs=[ag_out[:].opt()])
                    # gathered raw yT -> xT (one DMA, rank-major -> column blocks)
                    nc.sync.dma_start(
                        out=xT[:].rearrange("p (r c) -> p r c", r=NCORES),
                        in_=ag_out[:, :, 0:NO].rearrange("r p c -> p r c"))
                    st3 = wp.tile([C, NCORES, 2], f32r, name=f"st3_{li}", tag="st3")
                    nc.sync.dma_start(out=st3[:],
                                      in_=ag_out[:, :, NO:NO + 2].rearrange("r p c -> p r c"))
                    st3g = wp.tile([C, 2], f32, name=f"st3g_{li}", tag="st3g")
                    nc.vector.reduce_sum(st3g[:],
                                         st3[:].rearrange("p r c -> p c r"), axis=AX.X)
                    s3, t3 = bn_params(st3g[:, 0:1], st3g[:, 1:2],
                                       W[("n3g", i)][:], W[("n3b", i)][:], wp, f"bn3_{li}")
                    # BN3 applied in place on gathered xT and locally on own yT
                    nc.vector.tensor_scalar(out=xT[:], in0=xT[:], scalar1=s3[:],
                                            scalar2=t3[:], op0=OP.mult, op1=OP.add)
                    nc.vector.tensor_scalar(out=xq[:], in0=yT, scalar1=s3[:],
                                            scalar2=t3[:], op0=OP.mult, op1=OP.add)
                else:
                    # last layer: AR of BN3 stats only, then head MLP on own columns
                    st3 = wp.tile([C, 2], f32, name=f"st3_{li}", tag="st3f")
                    stats_into(yT, st3[:], f"y_{li}")
                    ar3_in = dp.tile([C, 2], f32, name="ar3_in", tag="ar3_in")
                    nc.sync.dma_start(out=ar3_in[:], in_=st3[:])
                    ar3_out = dp.tile([C, 2], f32, name="ar3_out", tag="ar3_out",
                                      addr_space="Shared")
                    nc.gpsimd.collective_compute("AllReduce", OP.add, replica_groups=RG,
                                                 ins=[ar3_in[:].opt()], outs=[ar3_out[:].opt()])
                    ar3_sb = wp.tile([C, 2], f32, name="ar3_sb", tag="ar3_sb")
                    nc.sync.dma_start(out=ar3_sb[:], in_=ar3_out[:])
                    s3, t3 = bn_params(ar3_sb[:, 0:1], ar3_sb[:, 1:2],
                                       W[("n3g", i)][:], W[("n3b", i)][:], wp, f"bn3_{li}")
                    xf = wp.tile([C, NO], f32r, name="xf", tag="xf")
                    nc.vector.tensor_scalar(out=xf[:], in0=yT, scalar1=s3[:],
                                            scalar2=t3[:], op0=OP.mult, op1=OP.add)
                    h1p = pmm.tile([C // 2, NO], f32, name="h1p", tag="mm")
                    nc.tensor.matmul(h1p[:], hw1[:], xf[:], start=True, stop=True)
                    hr1 = wp.tile([C // 2, NO], f32r, name="hr1", tag="hr1")
                    nc.scalar.activation(hr1[:], h1p[:], AF.Relu, bias=hb1[:], scale=1.0)
                    h2p = pmm.tile([C // 4, NO], f32, name="h2p", tag="mm")
                    nc.tensor.matmul(h2p[:], hw2[:], hr1[:], start=True, stop=True)
                    hr2 = wp.tile([C // 4, NO], f32r, name="hr2", tag="hr2")
                    nc.scalar.activation(hr2[:], h2p[:], AF.Relu, bias=hb2[:], scale=1.0)
                    h3p = pmm.tile([1, NO], f32, name="h3p", tag="mm")
                    nc.tensor.matmul(h3p[:], hw3[:], hr2[:], start=True, stop=True)
                    outs = wp.tile([1, NO], f32, name="outs", tag="outs")
                    nc.vector.tensor_scalar(out=outs[:], in0=h3p[:], scalar1=hb3[:],
                                            scalar2=None, op0=OP.add)
                    nc.sync.dma_start(out=d_out[:], in_=outs[:])

    nc.compile()
    return nc


def _host_prep(inputs):
    x = np.asarray(inputs["x"], dtype=np.float32)
    ei = np.asarray(inputs["edge_index"])
    src, dst = np.asarray(ei[0], dtype=np.int64), np.asarray(ei[1], dtype=np.int64)
    AT = np.zeros((N, N), dtype=np.float32)
    np.add.at(AT, (src, dst), 1.0)

    xT = np.ascontiguousarray(x.T)
    sd = 1.0 / np.sqrt(np.float32(D))

    common = {
        "xT": xT,
        "ident": np.eye(C, dtype=np.float32),
        "onesc": np.ones((C, C), dtype=np.float32),
        "hw1T": np.ascontiguousarray(np.asarray(inputs["head_w1"], np.float32).T),
        "hw2T": np.ascontiguousarray(np.asarray(inputs["head_w2"], np.float32).T),
        "hw3T": np.ascontiguousarray(np.asarray(inputs["head_w3"], np.float32).T),
        "hb1": np.asarray(inputs["head_b1"], np.float32).reshape(-1, 1),
        "hb2": np.asarray(inputs["head_b2"], np.float32).reshape(-1, 1),
        "hb3": np.asarray(inputs["head_b3"], np.float32).reshape(-1, 1),
    }
    wiT = np.stack([np.asarray(inputs["attn_in_w"][i], np.float32).T for i in range(L)])
    wib = np.stack([np.ascontiguousarray(
        np.asarray(inputs["attn_in_b"][i], np.float32).reshape(3, C).T)
        for i in range(L)])
    wiT = wiT.copy()
    wib = wib.copy()
    wiT[:, :, 0:C] *= sd          # fold 1/sqrt(d) into q projection
    wib[:, :, 0] *= sd
    # head-3 q/k duplicates at partition base 0 (PE quadrant-3 workaround)
    wiT = np.concatenate([wiT,
                          wiT[:, :, C + 3 * D:C + 4 * D],        # k head3
                          wiT[:, :, 3 * D:4 * D]], axis=2)        # q head3 (scaled)
    wib3 = np.zeros((L, C, 2), dtype=np.float32)
    wib3[:, 0:D, 0] = wib[:, 3 * D:4 * D, 1]                      # k head3 bias
    wib3[:, 0:D, 1] = wib[:, 3 * D:4 * D, 0]                      # q head3 bias (scaled)
    wib = np.concatenate([wib, wib3], axis=2)
    woTh = np.zeros((L, D, 4 * C), dtype=np.float32)
    for i in range(L):
        woT = np.asarray(inputs["attn_out_w"][i], np.float32).T   # [C, C]
        for h in range(HEADS):
            woTh[i, :, h * C:(h + 1) * C] = woT[h * D:(h + 1) * D, :]
    common.update({
        "wiT": wiT, "wib": wib, "woTh": woTh,
        "wob": np.stack([np.asarray(inputs["attn_out_b"][i], np.float32).reshape(-1, 1)
                         for i in range(L)]),
        "g1T": np.stack([np.asarray(inputs["gin_w1"][i], np.float32).T for i in range(L)]),
        "g1b": np.stack([np.asarray(inputs["gin_b1"][i], np.float32).reshape(-1, 1)
                         for i in range(L)]),
        "g2T": np.stack([np.asarray(inputs["gin_w2"][i], np.float32).T for i in range(L)]),
        "g2b": np.stack([np.asarray(inputs["gin_b2"][i], np.float32).reshape(-1, 1)
                         for i in range(L)]),
        "m1T": np.stack([np.asarray(inputs["mlp_w1"][i], np.float32).T for i in range(L)]),
        "m1b": np.stack([np.ascontiguousarray(
            np.asarray(inputs["mlp_b1"][i], np.float32).reshape(2, C).T)
            for i in range(L)]),
        "m2T": np.stack([np.ascontiguousarray(
            np.asarray(inputs["mlp_w2"][i], np.float32).T.reshape(2, C, C).transpose(1, 0, 2))
            for i in range(L)]),
        "m2b": np.stack([np.asarray(inputs["mlp_b2"][i], np.float32).reshape(-1, 1)
                         for i in range(L)]),
        "bng": np.stack([np.stack([np.asarray(inputs[k][i], np.float32).reshape(-1, 1)
                                   for k in ("n1_g", "n2_g", "n3_g")]) for i in range(L)]),
        "bnb": np.stack([np.stack([np.asarray(inputs[k][i], np.float32).reshape(-1, 1)
                                   for k in ("n1_b", "n2_b", "n3_b")]) for i in range(L)]),
    })
    in_maps = []
    for r in range(NCORES):
        m = dict(common)
        m["xq"] = np.ascontiguousarray(xT[:, r * NO:(r + 1) * NO])
        m["AT"] = np.ascontiguousarray(AT[:, r * NO:(r + 1) * NO])
        in_maps.append(m)
    return in_maps


def kernel(**inputs):
    if "nc" not in _CACHED:
        _CACHED["nc"] = _build()
    nc = _CACHED["nc"]
    in_maps = _host_prep(inputs)
    res = bass_utils.run_bass_kernel_spmd(nc, in_maps, core_ids=list(range(NCORES)))
    y = np.zeros((N, 1), dtype=np.float32)
    for r in range(NCORES):
        y[r * NO:(r + 1) * NO, 0] = res.results[r]["out"][0]
    return y
